# revision 10
# baseline (speedup 1.0000x reference)
"""DSSM (diagonal state space model) Trainium2 kernel.

Takes FULL inputs, returns FULL output; internally H-sharded over 8
NeuronCores (32 channels per core).  Algorithm: chunked semiseparable scan.

Per channel h the reference computes y = causal_conv(K_h, u) + D_h*u, masked
by per-batch length, where K_h[s] = 2*Re(sum_n Ct_hn * lam_hn^s).
L=4096 is split into 16 chunks of Q=256 (2 time-blocks of 128):
  - intra-chunk: block-Toeplitz matmuls with TK0/TK1 (D folded into diag)
  - inter-chunk: complex state x_c = lam^Q x_{c-1} + G @ u_chunk (gather
    matmul + 15-step DVE recurrence), then y += S @ x_{c-1} (scatter matmul)
All matmuls run in float32r (~1.5e-4 rel err, 4x faster than fp32 on PE).
Host precomputes the per-channel tables (tiny: O(H*(N+P)*P)).
"""
import os
import numpy as np

import concourse.bacc as bacc
import concourse.mybir as mybir
from concourse.tile import TileContext
from concourse import bass_utils, masks

B, H, N, L = 16, 256, 64, 4096
NCORES = 8
HL = H // NCORES          # 32 channels per core
P = 128                   # time block / partition size
Q = 256                   # chunk length
NCH = L // Q              # 16 chunks
NBLK = L // P             # 32 time blocks
CW = NCH + 1              # 17 chunk-column slots per h in the state tensor
F32 = mybir.dt.float32
F32R = mybir.dt.float32r

LAST_RESULT = None        # test.py reads exec_time_ns from here
_NC_CACHE = None


def _install_ntff_hook():
    """Optional: register the axon NTFF profiling hook so BASS_TRACE=1 works."""
    import sys, types
    try:
        if 'antenv.axon_hooks' in sys.modules:
            return
        import antenv
        mod = types.ModuleType('antenv.axon_hooks')
        mod._hook = None
        mod.set_axon_ntff_profile_hook = lambda h: setattr(mod, '_hook', h)
        mod.get_axon_ntff_profile_hook = lambda: mod._hook
        sys.modules['antenv.axon_hooks'] = mod
        antenv.axon_hooks = mod
        from trn_agent_boot.trn_boot import _ntff_profile_via_ctypes
        hook = _ntff_profile_via_ctypes('/opt/axon/libaxon_pjrt.so')
        if hook is not None:
            mod.set_axon_ntff_profile_hook(hook)
    except Exception:
        pass


def make_tables(log_dt, C, log_A_real, A_imag, D, length):
    """Per-channel fp32 tables (full H), complex128 internally."""
    dt = np.exp(log_dt.astype(np.float64))                                   # (H,)
    A = -np.exp(log_A_real.astype(np.float64)) + 1j * A_imag.astype(np.float64)
    dtA = A * dt[:, None]
    lam = np.exp(dtA)                                                        # (H,N)
    Ct = (C[..., 0] + 1j * C[..., 1]).astype(np.complex128) * (np.exp(dtA) - 1.0) / A

    s = np.arange(2 * P)
    lam_pow = lam[:, :, None] ** s[None, None, :]                            # (H,N,2P)
    K = 2.0 * np.einsum('hn,hns->hs', Ct, lam_pow).real                      # (H,2P)

    i = np.arange(P)
    d_ij = i[:, None] - i[None, :]
    T0 = np.where(d_ij >= 0, K[:, d_ij], 0.0)                                # (H,P,P)
    T0[:, i, i] = K[:, 0][:, None] + D[:, None].astype(np.float64)
    T1 = K[:, P + d_ij]                                                      # (H,P,P)
    tk0 = np.ascontiguousarray(np.swapaxes(T0, 1, 2)).astype(np.float32)     # lhsT
    tk1 = np.ascontiguousarray(np.swapaxes(T1, 1, 2)).astype(np.float32)

    j = np.arange(Q)
    g = lam[:, :, None] ** (Q - 1 - j)[None, None, :]                        # (H,N,Q)
    Gr = np.concatenate([g.real, g.imag], axis=1)                            # (H,2N,Q)
    gt = np.ascontiguousarray(np.swapaxes(Gr, 1, 2)).astype(np.float32)      # (H,Q,2N)

    tau = np.arange(Q)
    w = 2.0 * Ct[:, None, :] * lam[:, None, :] ** (tau + 1)[None, :, None]   # (H,Q,N)
    # st [H, 64, 512]: cols ri*256 + blk*128 + i ; lhsT rows = n
    wre = np.swapaxes(w.real, 1, 2)                                          # (H,N,Q)
    wim = np.swapaxes(-w.imag, 1, 2)                                         # (H,N,Q)
    st = np.concatenate([wre, wim], axis=2).astype(np.float32)               # (H,N,2Q)

    lamQ = lam ** Q                                                          # (H,N)
    ar = lamQ.real.astype(np.float32)
    ai = lamQ.imag.astype(np.float32)
    # fact [128, 512] per core: cols 0:256 = FR (ar), 256:512 = FI (ai);
    # partition = 64*hq + n (h = hq*16 + h2), free col = h2*16 + b
    facts = []
    for k in range(NCORES):
        arr = np.zeros((128, 512), np.float32)
        for hq in range(2):
            hs = ar[k * HL + hq * 16: k * HL + (hq + 1) * 16]      # (16,N)
            hi = ai[k * HL + hq * 16: k * HL + (hq + 1) * 16]
            arr[64 * hq:64 * hq + 64, 0:256] = np.repeat(
                hs.T[:, :, None], B, axis=2).reshape(N, 16 * B)
            arr[64 * hq:64 * hq + 64, 256:512] = np.repeat(
                hi.T[:, :, None], B, axis=2).reshape(N, 16 * B)
        facts.append(arr)

    # mask [128, 512]: [:,0:256] blk0 of each chunk, [:,256:512] blk1; col=c*16+b
    ln = length.astype(np.int64)[None, None, :]                              # (1,1,B)
    c_idx = np.arange(NCH)[None, :, None]
    i_idx = np.arange(P)[:, None, None]
    m0 = (((2 * c_idx) * P + i_idx) < ln).astype(np.float32)                 # (P,NCH,B)
    m1 = (((2 * c_idx + 1) * P + i_idx) < ln).astype(np.float32)
    maskt = np.concatenate([m0.reshape(P, NCH * B), m1.reshape(P, NCH * B)], axis=1)

    return tk0, tk1, gt, st, facts, maskt


def build_nc():
    """Build the (shared) per-core Bass program."""
    nc = bacc.Bacc()
    u_d = nc.dram_tensor("u", [B, HL, L], F32, kind="ExternalInput")
    tk0_d = nc.dram_tensor("tk0", [HL, P, P], F32R, kind="ExternalInput")
    tk1_d = nc.dram_tensor("tk1", [HL, P, P], F32R, kind="ExternalInput")
    gt_d = nc.dram_tensor("gt", [HL, Q, P], F32R, kind="ExternalInput")
    st_d = nc.dram_tensor("st", [HL, N, 2 * Q], F32R, kind="ExternalInput")
    fact_d = nc.dram_tensor("fact", [128, 512], F32R, kind="ExternalInput")
    mask_d = nc.dram_tensor("mask", [128, 512], F32, kind="ExternalInput")
    y_d = nc.dram_tensor("y", [B, HL, L], F32, kind="ExternalOutput")

    with TileContext(nc) as tc, tc.tile_pool(name="const", bufs=1) as constp:
        ident = constp.tile([P, P], F32)
        masks.make_identity(nc, ident[:])
        fact = constp.tile([128, 512], F32R)
        nc.sync.dma_start(out=fact[:], in_=fact_d[:])
        maskt = constp.tile([128, 512], F32)
        nc.sync.dma_start(out=maskt[:], in_=mask_d[:])
        # big persistent tensors
        ut = constp.tile([P, HL * NBLK * B], F32R, tag="ut")        # 16384 cols
        xls = constp.tile([P, HL * CW * B], F32R, tag="xls")        # 8704 cols

        # views
        # ut col = h*512 + Cblk*16 + b ; (Cblk = c*2 + two)
        ut_qv = ut[:].rearrange("p (h q r) -> p (h q) r", h=HL, q=4, r=P)
        ut_c2 = ut[:].rearrange("p (h c two b) -> p h two c b",
                                h=HL, c=NCH, two=2, b=B)
        # xls: partition = 64*hq + n ; col = h2*544 + ri*272 + cc*16 + b
        xls_v = xls[:].rearrange("p (h2 ri cc b) -> p h2 ri cc b",
                                 h2=16, ri=2, cc=CW, b=B)

        # ---- phase A: load u, transpose to time-major ----
        with tc.tile_pool(name="stage_a", bufs=3) as pa, \
             tc.tile_pool(name="psum_a", bufs=3, space="PSUM") as psa:
            for b in range(B):
                ust = pa.tile([P, 1024], F32, tag="ust")
                # u[b] is contiguous (HL*L); partition p = h*4+q, free = t in quarter
                nc.sync.dma_start(
                    out=ust[:], in_=u_d[b].rearrange("h (q t) -> (h q) t", q=4))
                for s in range(8):
                    pt = psa.tile([P, P], F32, tag="pt")
                    nc.tensor.transpose(pt[:], ust[:, s * P:(s + 1) * P], ident[:])
                    # src free = (h 32, q 4); dest col = (h*4+q)*128 + (s*16+b)
                    nc.scalar.copy(out=ut_qv[:, :, s * 16 + b], in_=pt[:])

        # ---- phase B: gather matmuls -> local states v[c] into xls (cc=c+1) ----
        # zero the cc=0 column block (x[-1] = 0) for every h
        zcol = constp.tile([P, B], F32)
        nc.gpsimd.memset(zcol[:], 0.0)
        for _h2 in range(16):
            for _ri in range(2):
                nc.scalar.copy(out=xls_v[:, _h2, _ri, 0, :], in_=zcol[:])

        with tc.tile_pool(name="par_b", bufs=4) as pb, \
             tc.tile_pool(name="psum_b", bufs=3, space="PSUM") as psb:
            for h in range(HL):
                hq, h2 = divmod(h, 16)
                gtt = pb.tile([P, 2 * P], F32R, tag="gtt")
                nc.sync.dma_start(out=gtt[:, 0:P], in_=gt_d[h][0:P])
                nc.sync.dma_start(out=gtt[:, P:2 * P], in_=gt_d[h][P:Q])
                xlp = psb.tile([P, NCH * B], F32, tag="xlp")
                nc.tensor.matmul(xlp[:], gtt[:, 0:P], ut_c2[:, h, 0],
                                 start=True, stop=False)
                nc.tensor.matmul(xlp[:], gtt[:, P:2 * P], ut_c2[:, h, 1],
                                 start=False, stop=True)
                rows = slice(64 * hq, 64 * hq + 64)
                nc.scalar.copy(out=xls_v[rows, h2, 0, 1:CW, :], in_=xlp[0:64, :])
                nc.scalar.copy(out=xls_v[rows, h2, 1, 1:CW, :], in_=xlp[64:128, :])

        # ---- phase C: inter-chunk recurrence on DVE (all ops base-0, 128p) ----
        FR = fact[:, 0:256]
        FI = fact[:, 256:512]
        with tc.tile_pool(name="rec", bufs=2) as pr:
            for c in range(1, NCH):
                xre = xls_v[:, :, 0, c, :]      # [128, (h2,b)=256]  re x[c-1]
                xim = xls_v[:, :, 1, c, :]
                nre = xls_v[:, :, 0, c + 1, :]  # v[c] -> x[c]
                nim = xls_v[:, :, 1, c + 1, :]
                p1 = pr.tile([128, 256], F32R, tag="p1")
                p2 = pr.tile([128, 256], F32R, tag="p2")
                p3 = pr.tile([128, 256], F32R, tag="p3")
                p4 = pr.tile([128, 256], F32R, tag="p4")
                nc.vector.tensor_mul(p1[:], FR, xre)
                nc.vector.tensor_mul(p2[:], FI, xim)
                nc.vector.tensor_mul(p3[:], FI, xre)
                nc.vector.tensor_mul(p4[:], FR, xim)
                s1 = pr.tile([128, 256], F32R, tag="s1")
                s2 = pr.tile([128, 256], F32R, tag="s2")
                nc.vector.tensor_sub(s1[:], p1[:], p2[:])
                nc.vector.tensor_add(s2[:], p3[:], p4[:])
                nc.vector.tensor_add(nre, nre, s1[:])
                nc.vector.tensor_add(nim, nim, s2[:])

        # ---- phase D: intra + scatter matmuls, mask, transpose out, store ----
        with tc.tile_pool(name="par_d", bufs=4) as pd, \
             tc.tile_pool(name="ysb", bufs=3) as pysb, \
             tc.tile_pool(name="psum_d", bufs=2, space="PSUM") as psd:
            for h in range(HL):
                hq, h2 = divmod(h, 16)
                rows = slice(64 * hq, 64 * hq + 64)
                t0 = pd.tile([P, P], F32R, tag="t0")
                t1 = pd.tile([P, P], F32R, tag="t1")
                stt = pd.tile([P, 2 * Q], F32R, tag="stt")
                nc.sync.dma_start(out=t0[:], in_=tk0_d[h])
                nc.sync.dma_start(out=t1[:], in_=tk1_d[h])
                nc.sync.dma_start(out=stt[rows, :], in_=st_d[h])
                y0 = psd.tile([P, NCH * B], F32, tag="y0")
                y1 = psd.tile([P, NCH * B], F32, tag="y1")
                xsr = xls_v[rows, h2, 0, 0:NCH, :]   # [64, (cc,b)=256]
                xsi = xls_v[rows, h2, 1, 0:NCH, :]
                nc.tensor.matmul(y0[:], t0[:], ut_c2[:, h, 0], start=True, stop=False)
                nc.tensor.matmul(y1[:], t0[:], ut_c2[:, h, 1], start=True, stop=False)
                nc.tensor.matmul(y1[:], t1[:], ut_c2[:, h, 0], start=False, stop=False)
                nc.tensor.matmul(y0[:], stt[rows, 0:128], xsr, start=False, stop=False)
                nc.tensor.matmul(y0[:], stt[rows, 256:384], xsi, start=False, stop=True)
                nc.tensor.matmul(y1[:], stt[rows, 128:256], xsr, start=False, stop=False)
                nc.tensor.matmul(y1[:], stt[rows, 384:512], xsi, start=False, stop=True)
                ysb = pysb.tile([P, NBLK * B], F32, tag="ysbt")
                ysb_v = ysb[:].rearrange("p (c two b) -> p two c b",
                                         c=NCH, two=2, b=B)
                nc.vector.tensor_mul(ysb_v[:, 0], y0[:], maskt[:, 0:256])
                nc.vector.tensor_mul(ysb_v[:, 1], y1[:], maskt[:, 256:512])
                # transpose out: 4 groups of 128 cols -> [(cl b), i]
                y_t = y_d[:, h, :].rearrange(
                    "b (g cl i) -> g cl b i", g=4, cl=8, i=P)
                for g in range(4):
                    pto = psd.tile([P, P], F32, tag="pto")
                    nc.tensor.transpose(pto[:], ysb[:, g * P:(g + 1) * P], ident[:])
                    yo = pysb.tile([P, P], F32, tag="yot")
                    nc.scalar.copy(out=yo[:], in_=pto[:])
                    nc.sync.dma_start(out=y_t[g], in_=yo[:])

    nc.finalize()
    return nc


def _get_nc():
    global _NC_CACHE
    if _NC_CACHE is None:
        _NC_CACHE = build_nc()
    return _NC_CACHE


def kernel(u, log_dt, C, log_A_real, A_imag, D, length):
    global LAST_RESULT
    u = np.ascontiguousarray(np.asarray(u, dtype=np.float32))
    tk0, tk1, gt, st, facts, maskt = make_tables(
        np.asarray(log_dt), np.asarray(C), np.asarray(log_A_real),
        np.asarray(A_imag), np.asarray(D), np.asarray(length))

    if os.environ.get("BASS_TRACE"):
        _install_ntff_hook()
    nc = _get_nc()

    in_maps = []
    for k in range(NCORES):
        hs = slice(k * HL, (k + 1) * HL)
        in_maps.append({
            "u": np.ascontiguousarray(u[:, hs, :]),
            "tk0": tk0[hs], "tk1": tk1[hs], "gt": gt[hs], "st": st[hs],
            "fact": facts[k], "mask": maskt,
        })
    res = bass_utils.run_bass_kernel_spmd(nc, in_maps, core_ids=list(range(NCORES)))
    LAST_RESULT = res
    y = np.concatenate([res.results[k]["y"] for k in range(NCORES)], axis=1)
    return np.ascontiguousarray(y)
